# revision 39
# baseline (speedup 1.0000x reference)
"""Trainium2 Bass kernel for a dense transformer block (RoPE attention + SwiGLU).

Sharding (8 NeuronCores, Megatron-style):
  - QKV + attention: tensor-parallel over heads (2 heads/core, both batches).
  - Two AllToAlls (one per local head) reshard attention output from
    head-sharded to token-sharded; the first is issued halfway through
    attention so it overlaps with the second head's compute.
  - proj + SwiGLU MLP: token-sharded (512 tokens/core), fully local.
Host pre-transposes x and all weights so every matmul contracts over the
partition axis. RoPE's half-swap is done on device with a permutation
matmul (avoids duplicating q/k columns in the QKV GEMM). The whole
attention phase runs in (64,128) PE-tiling mode (scores contract over
d=64; attn@v splits its 128-token contraction across the two row tiles),
so there are no PE mode-switch drains inside the phase.
Softmax: exp on ScalarE in FD=1024 chunks (the phase pacer); the
denominator reciprocal runs on DVE off the critical path, and is
broadcast across partitions with a ones-row matmul whose rhs row is
staged at partition 0 by a small DMA. The normalize matmul is emitted
one slot late so the in-order tensor queue never waits on the vector
chain. All matmuls run in bf16 with fp32 PSUM accumulation.
"""

import functools
import numpy as np
import ml_dtypes

B, T, C, H, D = 2, 2048, 1024, 16, 64
HID = 4 * C
NCORES = 8
HPC = H // NCORES          # heads per core


def _build_program(b, t):
    import concourse.bacc as bacc
    import concourse.mybir as mybir
    import concourse.tile as tile
    import concourse.masks as masks
    from contextlib import ExitStack

    fp32 = mybir.dt.float32
    bf16 = mybir.dt.bfloat16
    Act = mybir.ActivationFunctionType
    Alu = mybir.AluOpType

    tok = b * t                    # all tokens (b-major)
    tpc = tok // NCORES            # tokens per core for proj/MLP/out
    m_qkv = 3 * HPC * D            # q, k, v local cols (384)
    kt_tiles = t // 128            # 128-token key tiles per (b,h) unit
    kt2 = kt_tiles // 2
    assert kt_tiles % 2 == 0
    qt_chunk = min(512, t)
    qt_chunks = t // qt_chunk
    n_chunk = min(512, tok)
    n_chunks = tok // n_chunk
    ck = C // 128                  # C chunks (8)
    mh_tiles = HID // 128          # hidden chunks (32)
    hg = 8                         # hidden chunks per weight-stream group
    scale = float(D) ** -0.5

    nc = bacc.Bacc("TRN2", target_bir_lowering=False, debug=False,
                   num_devices=NCORES)

    # ---- DRAM I/O ----
    xT_d = nc.dram_tensor("xT", [C, tok], bf16, kind="ExternalInput")
    wqkvT_d = nc.dram_tensor("wqkvT", [C, m_qkv], bf16, kind="ExternalInput")
    bqkv_d = nc.dram_tensor("bqkv2d", [128, 3], fp32, kind="ExternalInput")
    psw_d = nc.dram_tensor("pswd", [128, 128], bf16, kind="ExternalInput")
    cos_d = nc.dram_tensor("cosd", [128, tok], bf16, kind="ExternalInput")
    sin_d = nc.dram_tensor("sind", [128, tok], bf16, kind="ExternalInput")
    wprojT_d = nc.dram_tensor("wprojT", [C, C], bf16, kind="ExternalInput")
    bproj_d = nc.dram_tensor("bproj2d", [128, ck], fp32, kind="ExternalInput")
    w1T_d = nc.dram_tensor("w1T", [C, HID], bf16, kind="ExternalInput")
    w2T_d = nc.dram_tensor("w2T", [C, HID], bf16, kind="ExternalInput")
    w3T_d = nc.dram_tensor("w3T", [HID, C], bf16, kind="ExternalInput")
    b1_d = nc.dram_tensor("b1_2d", [128, mh_tiles], fp32, kind="ExternalInput")
    b2_d = nc.dram_tensor("b2_2d", [128, mh_tiles], fp32, kind="ExternalInput")
    b3_d = nc.dram_tensor("b3_2d", [128, ck], fp32, kind="ExternalInput")
    y_d = nc.dram_tensor("y_loc", [C, tpc], fp32, kind="ExternalOutput")

    with tile.TileContext(nc) as tc:
        es = ExitStack()
        # ---- constants / biases (live whole kernel) ----
        consts = es.enter_context(tc.tile_pool(name="consts", bufs=1))
        ident = consts.tile([128, 128], bf16, name="ident")
        masks.make_identity(nc, ident[:])
        # broadcast stationary: row 0 ones, rows 1-63 zero -> MM replicates
        # the rhs row-0 reciprocal across 65 output partitions in-mode.
        onepad = consts.tile([64, 65], bf16, name="onepad")
        nc.vector.memset(onepad[:], 0.0)
        nc.vector.memset(onepad[0:1, :], 1.0)
        psw_sb = consts.tile([128, 128], bf16, name="psw_sb")
        nc.sync.dma_start(out=psw_sb[:], in_=psw_d[:, :])
        bqkv_sb = consts.tile([128, 3], fp32, name="bqkv_sb")
        nc.sync.dma_start(out=bqkv_sb[:], in_=bqkv_d[:, :])
        bproj_sb = consts.tile([128, ck], fp32, name="bproj_sb")
        nc.sync.dma_start(out=bproj_sb[:], in_=bproj_d[:, :])
        b1_sb = consts.tile([128, mh_tiles], fp32, name="b1_sb")
        nc.sync.dma_start(out=b1_sb[:], in_=b1_d[:, :])
        b2_sb = consts.tile([128, mh_tiles], fp32, name="b2_sb")
        nc.sync.dma_start(out=b2_sb[:], in_=b2_d[:, :])
        b3_sb = consts.tile([128, ck], fp32, name="b3_sb")
        nc.sync.dma_start(out=b3_sb[:], in_=b3_d[:, :])

        # ---- attention-lifetime tensors ----
        attn_pool = es.enter_context(tc.tile_pool(name="attn", bufs=1))
        qr = attn_pool.tile([128, tok], bf16, name="qr")
        kr = attn_pool.tile([128, tok], bf16, name="kr")
        vaug_cols = 65 * kt_tiles * b * HPC
        v_aug = attn_pool.tile([128, vaug_cols], bf16, name="v_aug")
        nc.vector.memset(v_aug[:], 1.0)
        outT_h0 = attn_pool.tile([64, tok], bf16, name="outT_h0")
        outT_h1 = attn_pool.tile([64, tok], bf16, name="outT_h1")

        # ---- A2A bounce buffers (one pair per local head) ----
        dram = es.enter_context(tc.tile_pool(name="dramp", bufs=1,
                                             space="DRAM"))
        a2a_in = []
        a2a_out = []
        for hh in range(HPC):
            ai = dram.tile([NCORES * 64, tpc], bf16, name=f"a2a_in{hh}")
            ao = dram.tile([NCORES * 64, tpc], bf16, name=f"a2a_out{hh}")
            a2a_in.append(ai)
            a2a_out.append(ao)

        # ================= Phase A: QKV GEMM + RoPE + v transpose ======
        ph_a = ExitStack()
        xt_pool = ph_a.enter_context(tc.tile_pool(name="xt", bufs=1))
        wq_pool = ph_a.enter_context(tc.tile_pool(name="wq", bufs=1))
        qkv_sb_pool = ph_a.enter_context(tc.tile_pool(name="qkvsb", bufs=1))
        rope_tab = ph_a.enter_context(tc.tile_pool(name="ropetab", bufs=1))
        ps_qkv = ph_a.enter_context(
            tc.tile_pool(name="ps_qkv", bufs=3, space="PSUM"))
        ps_sw = ph_a.enter_context(
            tc.tile_pool(name="ps_sw", bufs=2, space="PSUM"))
        ps_tr = ph_a.enter_context(
            tc.tile_pool(name="ps_tr", bufs=2, space="PSUM"))

        wq_sb = []
        for kc in range(ck):
            wq_kc = wq_pool.tile([128, m_qkv], bf16, name=f"wqkv{kc}")
            nc.sync.dma_start(out=wq_kc[:],
                              in_=wqkvT_d[128 * kc:128 * kc + 128, :])
            wq_sb.append(wq_kc)
        # xT DMA'd in n-chunk column slices (n-outer) so the first QKV
        # matmul only waits for 1MB, not the full 8MB.  The RoPE tables
        # are queued behind the first two token chunks.
        xt_sb = []
        for kc in range(ck):
            xt_kc = xt_pool.tile([128, tok], bf16, name=f"xt{kc}")
            xt_sb.append(xt_kc)
        cos_sb = rope_tab.tile([128, tok], bf16, name="cos_sb")
        sin_sb = rope_tab.tile([128, tok], bf16, name="sin_sb")
        for n in range(n_chunks):
            c0 = n * n_chunk
            for kc in range(ck):
                nc.sync.dma_start(
                    out=xt_sb[kc][:, c0:c0 + n_chunk],
                    in_=xT_d[128 * kc:128 * kc + 128, c0:c0 + n_chunk])
            if n == min(1, n_chunks - 1):
                nc.sync.dma_start(out=cos_sb[:], in_=cos_d[:, :])
                nc.sync.dma_start(out=sin_sb[:], in_=sin_d[:, :])

        q_bf = qkv_sb_pool.tile([128, tok], bf16, name="q_bf")
        k_bf = qkv_sb_pool.tile([128, tok], bf16, name="k_bf")
        v_bf = qkv_sb_pool.tile([128, tok], bf16, name="v_bf")
        qtb = qkv_sb_pool.tile([128, tok], bf16, name="qtb")
        ktb = qkv_sb_pool.tile([128, tok], bf16, name="ktb")
        ta_scr = qkv_sb_pool.tile([128, tok], bf16, name="ta_scr")

        dest = [q_bf, k_bf, v_bf]
        for n in range(n_chunks):
            c0 = n * n_chunk
            cs = slice(c0, c0 + n_chunk)
            for mi in range(3):
                ps = ps_qkv.tile([128, n_chunk], fp32, name=f"psqkv{n}_{mi}",
                                 tag="psqkv")
                for kc in range(ck):
                    nc.tensor.matmul(
                        ps[:], wq_sb[kc][:, 128 * mi:128 * mi + 128],
                        xt_sb[kc][:, cs],
                        start=(kc == 0), stop=(kc == ck - 1))
                # bias add + cast to bf16 on DVE (PSUM source)
                nc.vector.tensor_scalar(
                    dest[mi][:, cs], ps[:],
                    bqkv_sb[:, mi:mi + 1], None, Alu.add)
            # full RoPE per chunk (vector work rides under the QKV matmuls):
            # dst = src*cos + swap(src)*sin, swap via permutation matmul
            for src, tb, dst in ((q_bf, qtb, qr), (k_bf, ktb, kr)):
                psx = ps_sw.tile([128, n_chunk], fp32, name=f"psw{n}",
                                 tag="psw")
                nc.tensor.matmul(psx[:], psw_sb[:], src[:, cs],
                                 start=True, stop=True)
                nc.vector.tensor_mul(tb[:, cs], psx[:], sin_sb[:, cs])
                nc.vector.tensor_mul(ta_scr[:, cs], src[:, cs],
                                     cos_sb[:, cs])
                nc.vector.tensor_add(dst[:, cs], ta_scr[:, cs], tb[:, cs])

        # v_aug[(h,bi)] blocks: [128 ktok, 64 d] + ones col (65 stride)
        # unit order is h-major so head 0 finishes first for the early A2A.
        for h in range(HPC):
            for bi in range(b):
                u = h * b + bi
                base = u * 65 * kt_tiles
                for kt in range(kt_tiles):
                    pst = ps_tr.tile([128, 64], bf16, name=f"pst{u}_{kt}",
                                     tag="pst")
                    nc.tensor.transpose(
                        pst[:],
                        v_bf[64 * h:64 * h + 64,
                             bi * t + 128 * kt:bi * t + 128 * kt + 128],
                        ident[64 * h:64 * h + 64, 64 * h:64 * h + 64])
                    nc.vector.tensor_copy(
                        v_aug[:, base + 65 * kt:base + 65 * kt + 64], pst[:])

        ph_a.close()

        # ---- pools opened between phases: prefetched weights + aT ----
        es2 = ExitStack()
        wp_pool = es2.enter_context(tc.tile_pool(name="wpp", bufs=1))
        of_pool = es2.enter_context(tc.tile_pool(name="ofp", bufs=1))
        mlp_pool = es2.enter_context(tc.tile_pool(name="mlp", bufs=1))
        w1g_pool = es2.enter_context(tc.tile_pool(name="w1g", bufs=2))
        w2g_pool = es2.enter_context(tc.tile_pool(name="w2g", bufs=2))
        aT = mlp_pool.tile([128, ck * tpc], bf16, name="aT")

        wp_sb = []
        for kc in range(ck):
            wp_kc = wp_pool.tile([128, C], bf16, name=f"wp{kc}")
            nc.sync.dma_start(out=wp_kc[:],
                              in_=wprojT_d[128 * kc:128 * kc + 128, :])
            wp_sb.append(wp_kc)

        def load_mlp_group(g, defer=None):
            w1g = []
            w2g = []
            for kc in range(ck):
                w1k = w1g_pool.tile([128, hg * 128], bf16,
                                    name=f"w1g{g}_{kc}", tag=f"w1g{kc}")
                w1g.append(w1k)
                w2k = w2g_pool.tile([128, hg * 128], bf16,
                                    name=f"w2g{g}_{kc}", tag=f"w2g{kc}")
                w2g.append(w2k)

                def dma(w1k=w1k, w2k=w2k, g=g, kc=kc):
                    nc.sync.dma_start(
                        out=w1k[:],
                        in_=w1T_d[128 * kc:128 * kc + 128,
                                  hg * 128 * g:hg * 128 * (g + 1)])
                    nc.sync.dma_start(
                        out=w2k[:],
                        in_=w2T_d[128 * kc:128 * kc + 128,
                                  hg * 128 * g:hg * 128 * (g + 1)])

                if defer is None:
                    dma()
                else:
                    defer.append(dma)
            return w1g, w2g

        # both buffered groups prefetch mid-attention (emitted after the
        # first collective fires, so the h=0 slots' small per-slot DMAs
        # never ring-block behind 8MB of weights)
        mlp_groups = {}
        pending_dma = []

        # ================= Phase C: attention (pipelined) =================
        ph_c = ExitStack()
        ps_s = ph_c.enter_context(tc.tile_pool(name="ps_s", bufs=2,
                                               space="PSUM"))
        ps_o = ph_c.enter_context(tc.tile_pool(name="ps_o", bufs=2,
                                               space="PSUM"))
        exp_pool = ph_c.enter_context(tc.tile_pool(name="expp", bufs=11))
        sm_pool = ph_c.enter_context(tc.tile_pool(name="smp", bufs=2))
        i16_pool = ph_c.enter_context(tc.tile_pool(name="i16p", bufs=2))
        ib_pool = ph_c.enter_context(tc.tile_pool(name="ibp", bufs=2))
        ss_pool = ph_c.enter_context(tc.tile_pool(name="ssp", bufs=2))

        # slots: h-major so h=0 completes first
        slots = [(h, bi, qc) for h in range(HPC) for bi in range(b)
                 for qc in range(qt_chunks)]
        nslots = len(slots)
        h0_last = nslots // 2 - 1    # index of last h=0 slot
        state = {}  # si -> (exp chunks, psoA, psoB)

        def a2a_emit(hh):
            nc.gpsimd.collective_compute(
                "AllToAll", Alu.bypass,
                replica_groups=[list(range(NCORES))],
                ins=[a2a_in[hh][:]], outs=[a2a_out[hh][:]])

        def of_load(hh):
            # kept out of the attention window: these DMAs wait on the
            # collective and would head-of-line-block the sync DMA queue
            for kc in range(ck // 2):
                of_kc = of_pool.tile([128, tpc], bf16, name=f"of{hh}_{kc}")
                nc.sync.dma_start(
                    out=of_kc[:],
                    in_=a2a_out[hh][128 * kc:128 * kc + 128, :])
                of_sb.append(of_kc)

        of_sb = []

        def sc_step(si, kp):
            h, bi, qc = slots[si]
            q0 = bi * t + qc * qt_chunk
            krows = kr[64 * h:64 * h + 64, :]
            qrows = qr[64 * h:64 * h + 64, :]
            ps = ps_s.tile([128, 2 * qt_chunk], fp32,
                           name=f"pss{si}_{kp}", tag="pss")
            for j in (0, 1):
                kt = 2 * kp + j
                nc.tensor.matmul(
                    ps[:, j * qt_chunk:(j + 1) * qt_chunk],
                    krows[:, bi * t + 128 * kt:bi * t + 128 * kt + 128],
                    qrows[:, q0:q0 + qt_chunk],
                    start=True, stop=True)
            ex = exp_pool.tile([128, 2 * qt_chunk], bf16,
                               name=f"ex{si}_{kp}", tag="ex")
            nc.scalar.activation(ex[:], ps[:], Act.Exp, scale=scale)
            state[si][0].append(ex)

        def av_step(si, kp):
            h, bi, qc = slots[si]
            u = h * b + bi
            vbase = u * 65 * kt_tiles
            if kp == 0:
                psoA = ps_o.tile([65, qt_chunk], fp32, name=f"psoA{si}",
                                 tag="psoA")
                psoB = ps_o.tile([65, qt_chunk], fp32, name=f"psoB{si}",
                                 tag="psoB")
                state[si] = (state[si][0], psoA, psoB)
            _, psoA, psoB = state[si]
            ex = state[si][0][kp]
            for j in (0, 1):
                kt = 2 * kp + j
                vcols = v_aug[:, vbase + 65 * kt:vbase + 65 * kt + 65]
                exj = ex[:, j * qt_chunk:(j + 1) * qt_chunk]
                nc.tensor.matmul(psoA[:], vcols[0:64, :], exj[0:64, :],
                                 start=(kt == 0), stop=(kt == kt_tiles - 1))
                nc.tensor.matmul(psoB[:], vcols[64:128, :], exj[64:128, :],
                                 start=(kt == 0), stop=(kt == kt_tiles - 1))

        tail_state = {}

        def tail_a(si):
            # vector chain: sum the two pso halves, reciprocal of the
            # denominator row, stage the bf16 reciprocal at partition 0
            _, psoA, psoB = state[si]
            sB = ss_pool.tile([65, qt_chunk], fp32, name=f"sB{si}", tag="sB")
            nc.vector.tensor_copy(sB[:], psoB[:])
            ssum = ss_pool.tile([65, qt_chunk], fp32, name=f"ss{si}",
                                tag="ss")
            nc.vector.tensor_add(ssum[:], psoA[:], sB[:])
            inv = sm_pool.tile([65, qt_chunk], fp32, name=f"inv{si}",
                               tag="inv")
            nc.vector.reciprocal(inv[64:65, :], ssum[64:65, :])
            i16 = i16_pool.tile([65, qt_chunk], bf16, name=f"i16{si}",
                                tag="i16")
            nc.vector.tensor_copy(i16[64:65, :], inv[64:65, :])
            invb = ib_pool.tile([64, qt_chunk], bf16, name=f"ib{si}",
                                tag="ib")
            nc.vector.memset(invb[:], 0.0)
            # move the reciprocal row from partition 64 to partition 0
            nc.sync.dma_start(out=invb[0:1, :], in_=i16[64:65, :])
            tail_state[si] = (ssum, invb)
            state.pop(si)

        def tail_b(si):
            # emitted one slot later so the broadcast matmul's vector-chain
            # dependencies are long since ready (no tensor-queue stall)
            h, bi, qc = slots[si]
            ssum, invb = tail_state.pop(si)
            psb = ps_o.tile([65, qt_chunk], fp32, name=f"psb{si}", tag="psoB")
            nc.tensor.matmul(psb[:], onepad[:], invb[:], start=True,
                             stop=True)
            out_h = outT_h0 if h == 0 else outT_h1
            q0 = bi * t + qc * qt_chunk
            nc.vector.tensor_mul(out_h[:, q0:q0 + qt_chunk], ssum[0:64, :],
                                 psb[0:64, :])
            # this slot's columns cover whole A2A token-blocks: ship now
            assert qt_chunk % tpc == 0
            for j in range(q0 // tpc, (q0 + qt_chunk) // tpc):
                nc.sync.dma_start(out=a2a_in[h][64 * j:64 * j + 64, :],
                                  in_=out_h[:, tpc * j:tpc * (j + 1)])

        def open_slot(si):
            state[si] = ([], None, None)

        # software pipeline: scores of slot si interleave with attn@v of
        # si-1; the normalize matmul of si-2 rides along a further slot late
        open_slot(0)
        for kp in range(kt2):
            sc_step(0, kp)
        for si in range(1, nslots):
            open_slot(si)
            for kp in range(kt2):
                sc_step(si, kp)
                av_step(si - 1, kp)
            tail_a(si - 1)
            if si >= 2:
                tail_b(si - 2)
                if si - 2 == h0_last:
                    a2a_emit(0)
                    mlp_groups[0] = load_mlp_group(0, defer=pending_dma)
                    mlp_groups[1] = load_mlp_group(1, defer=pending_dma)
                else:
                    # dribble the deferred weight prefetch between slots so
                    # the per-slot DMAs never ring-block behind it
                    for _ in range(3):
                        if pending_dma:
                            pending_dma.pop(0)()
        for kp in range(kt2):
            av_step(nslots - 1, kp)
        tail_a(nslots - 1)
        tail_b(nslots - 2)
        tail_b(nslots - 1)
        a2a_emit(1)
        while pending_dma:
            pending_dma.pop(0)()
        of_load(0)
        of_load(1)

        ph_c.close()

        # ================= Phase E: out proj (token-sharded) ==========
        ph_e = ExitStack()
        ps_a = ph_e.enter_context(tc.tile_pool(name="ps_a", bufs=1,
                                               space="PSUM"))
        ps_am = []
        for m in range(ck):
            psa = ps_a.tile([128, tpc], fp32, name=f"psa{m}", tag=f"psa{m}")
            ps_am.append(psa)
        for kc in range(ck):
            for m in range(ck):
                nc.tensor.matmul(ps_am[m][:],
                                 wp_sb[kc][:, 128 * m:128 * m + 128],
                                 of_sb[kc][:],
                                 start=(kc == 0), stop=(kc == ck - 1))
        for m in range(ck):
            nc.scalar.activation(aT[:, tpc * m:tpc * (m + 1)], ps_am[m][:],
                                 Act.Identity, bias=bproj_sb[:, m:m + 1])
        ph_e.close()

        # ================= Phase F: SwiGLU MLP =================
        ph_f = ExitStack()
        ps_1 = ph_f.enter_context(tc.tile_pool(name="ps_1", bufs=2,
                                               space="PSUM"))
        ps_2 = ph_f.enter_context(tc.tile_pool(name="ps_2", bufs=2,
                                               space="PSUM"))
        s_pool = ph_f.enter_context(tc.tile_pool(name="sp", bufs=2))
        hT = mlp_pool.tile([128, mh_tiles * tpc], bf16, name="hT")

        ngroups = mh_tiles // hg
        for g in range(ngroups):
            w1g, w2g = mlp_groups.pop(g)
            if g + 2 < ngroups:
                mlp_groups[g + 2] = load_mlp_group(g + 2)
            for ml in range(hg):
                mh = hg * g + ml
                ps1 = ps_1.tile([128, tpc], fp32, name=f"ps1_{mh}", tag="ps1")
                ps2 = ps_2.tile([128, tpc], fp32, name=f"ps2_{mh}", tag="ps2")
                for kc in range(ck):
                    nc.tensor.matmul(ps1[:],
                                     w1g[kc][:, 128 * ml:128 * ml + 128],
                                     aT[:, tpc * kc:tpc * (kc + 1)],
                                     start=(kc == 0), stop=(kc == ck - 1))
                for kc in range(ck):
                    nc.tensor.matmul(ps2[:],
                                     w2g[kc][:, 128 * ml:128 * ml + 128],
                                     aT[:, tpc * kc:tpc * (kc + 1)],
                                     start=(kc == 0), stop=(kc == ck - 1))
                g_sb = s_pool.tile([128, tpc], fp32, name=f"g{mh}", tag="g")
                nc.scalar.activation(g_sb[:], ps1[:], Act.Sigmoid,
                                     bias=b1_sb[:, mh:mh + 1])
                s_sb = s_pool.tile([128, tpc], fp32, name=f"s{mh}", tag="s")
                nc.vector.scalar_tensor_tensor(
                    s_sb[:], ps1[:], b1_sb[:, mh:mh + 1], g_sb[:],
                    op0=Alu.add, op1=Alu.mult)
                nc.vector.scalar_tensor_tensor(
                    hT[:, tpc * mh:tpc * (mh + 1)], ps2[:],
                    b2_sb[:, mh:mh + 1], s_sb[:],
                    op0=Alu.add, op1=Alu.mult)
        ph_f.close()

        # ---- w3 ----
        ph_g = ExitStack()
        w3_pool = ph_g.enter_context(tc.tile_pool(name="w3p", bufs=3))
        ps_3 = ph_g.enter_context(tc.tile_pool(name="ps_3", bufs=1,
                                               space="PSUM"))
        out_pool = ph_g.enter_context(tc.tile_pool(name="outp", bufs=2))
        ps_3m = []
        for m in range(ck):
            ps3 = ps_3.tile([128, tpc], fp32, name=f"ps3_{m}", tag=f"ps3{m}")
            ps_3m.append(ps3)
        for kh in range(mh_tiles):
            w3k = w3_pool.tile([128, C], bf16, name=f"w3k{kh}", tag="w3k")
            nc.sync.dma_start(out=w3k[:],
                              in_=w3T_d[128 * kh:128 * kh + 128, :])
            for m in range(ck):
                nc.tensor.matmul(ps_3m[m][:],
                                 w3k[:, 128 * m:128 * m + 128],
                                 hT[:, tpc * kh:tpc * (kh + 1)],
                                 start=(kh == 0), stop=(kh == mh_tiles - 1))
        for m in range(ck):
            yo = out_pool.tile([128, tpc], fp32, name=f"yo{m}", tag="yo")
            nc.scalar.activation(yo[:], ps_3m[m][:], Act.Identity,
                                 bias=b3_sb[:, m:m + 1])
            nc.sync.dma_start(out=y_d[128 * m:128 * m + 128, :], in_=yo[:])
        ph_g.close()
        es2.close()
        es.close()

    nc.compile()
    return nc


@functools.lru_cache(maxsize=2)
def _get_program(b, t):
    return _build_program(b, t)


def _prep_inputs(x, w_qkv, b_qkv, w_proj, b_proj, w1, b1, w2, b2, w3, b3,
                 cos, sin, b, t):
    """Build per-core in_maps (host-side sharding / transposes / casts)."""
    bf = ml_dtypes.bfloat16
    tok = b * t
    ck = C // 128
    mh_tiles = HID // 128

    xT = np.ascontiguousarray(x.reshape(tok, C).T).astype(bf)
    # RoPE tables tiled to [128, tok]: rows = 4x the 32 freq rows,
    # cols = b-major tokens.
    cosd = np.tile(cos.T, (4, b)).astype(bf)
    sind = np.tile(np.concatenate([-sin.T, sin.T], axis=0), (2, b)).astype(bf)
    # half-swap permutation matrix: out row m <- in row swap(m)
    # (swap first/last 32 within each head's 64-dim block)
    psw = np.zeros((128, 128), dtype=np.float32)
    for m in range(128):
        j = m + 32 if (m % 64) < 32 else m - 32
        psw[j, m] = 1.0
    pswd = psw.astype(bf)
    # proj weight rows reordered: even global heads first, then odd
    # (matches the two per-head A2A result layouts)
    wpT = np.ascontiguousarray(w_proj.T)         # [H*D, C]
    head_order = list(range(0, H, 2)) + list(range(1, H, 2))
    wprojT = np.concatenate(
        [wpT[h * D:(h + 1) * D, :] for h in head_order], axis=0).astype(bf)
    bproj2d = np.ascontiguousarray(b_proj.reshape(ck, 128).T).astype(np.float32)
    w1T = np.ascontiguousarray(w1.T).astype(bf)
    w2T = np.ascontiguousarray(w2.T).astype(bf)
    w3T = np.ascontiguousarray(w3.T).astype(bf)
    b1_2d = np.ascontiguousarray(b1.reshape(mh_tiles, 128).T).astype(np.float32)
    b2_2d = np.ascontiguousarray(b2.reshape(mh_tiles, 128).T).astype(np.float32)
    b3_2d = np.ascontiguousarray(b3.reshape(ck, 128).T).astype(np.float32)

    # even/odd RoPE permutation within each head's 64 dims
    perm = np.concatenate([np.arange(0, D, 2), np.arange(1, D, 2)])

    in_maps = []
    for c in range(NCORES):
        rows = []
        brows = []
        secperm = [(0, perm), (1, perm), (2, np.arange(D))]
        for sec, p in secperm:        # q, k, v
            for hh in range(HPC):
                h = HPC * c + hh
                idx = sec * H * D + h * D + p
                rows.append(w_qkv[idx, :])
                brows.append(b_qkv[idx])
        wql = np.concatenate(rows, axis=0)           # [384, C]
        bql = np.concatenate(brows, axis=0)          # [384]
        wqkvT = np.ascontiguousarray(wql.T).astype(bf)
        bqkv2d = np.ascontiguousarray(bql.reshape(3, 128).T).astype(np.float32)
        in_maps.append({
            "xT": xT, "wqkvT": wqkvT, "bqkv2d": bqkv2d, "pswd": pswd,
            "cosd": cosd, "sind": sind,
            "wprojT": wprojT, "bproj2d": bproj2d,
            "w1T": w1T, "w2T": w2T, "w3T": w3T,
            "b1_2d": b1_2d, "b2_2d": b2_2d, "b3_2d": b3_2d,
        })
    return in_maps


def kernel(x, w_qkv, b_qkv, w_proj, b_proj, w1, b1, w2, b2, w3, b3, cos, sin,
           _trace=False):
    from concourse import bass_utils

    b, t, c = x.shape
    assert (b, t, c) == (B, T, C)
    args = [np.asarray(a, dtype=np.float32) for a in
            (x, w_qkv, b_qkv, w_proj, b_proj, w1, b1, w2, b2, w3, b3,
             cos, sin)]
    nc = _get_program(b, t)
    in_maps = _prep_inputs(*args, b, t)
    res = bass_utils.run_bass_kernel_spmd(
        nc, in_maps, core_ids=list(range(NCORES)), trace=_trace)
    tpc = (b * t) // NCORES
    y = np.empty((b * t, c), dtype=np.float32)
    for i in range(NCORES):
        y[tpc * i:tpc * (i + 1), :] = res.results[i]["y_loc"].T
    out = y.reshape(b, t, c)
    if _trace:
        return out, res
    return out


# revision 45
# speedup vs baseline: 1.0575x; 1.0575x over previous
"""Trainium2 Bass kernel for a dense transformer block (RoPE attention + SwiGLU).

Sharding (8 NeuronCores, Megatron-style):
  - QKV + attention: tensor-parallel over heads (2 heads/core, both batches).
  - Two AllToAlls (one per local head) reshard attention output from
    head-sharded to token-sharded; the first is issued halfway through
    attention so it overlaps with the second head's compute.
  - proj + SwiGLU MLP: token-sharded (512 tokens/core), fully local.
Host pre-transposes x and all weights so every matmul contracts over the
partition axis. RoPE's half-swap is done on device with a permutation
matmul (avoids duplicating q/k columns in the QKV GEMM). The whole
attention phase runs in (64,128) PE-tiling mode (scores contract over
d=64; attn@v splits its 128-token contraction across the two row tiles),
so there are no PE mode-switch drains inside the phase.
Softmax: exp on ScalarE in FD=1024 chunks (the phase pacer); the
denominator reciprocal runs on DVE off the critical path, and is
broadcast across partitions with a ones-row matmul whose rhs row is
staged at partition 0 by a small DMA. The normalize matmul is emitted
one slot late so the in-order tensor queue never waits on the vector
chain. All matmuls run in bf16 with fp32 PSUM accumulation.
"""

import functools
import numpy as np
import ml_dtypes

B, T, C, H, D = 2, 2048, 1024, 16, 64
HID = 4 * C
NCORES = 8
HPC = H // NCORES          # heads per core


def _build_program(b, t):
    import concourse.bacc as bacc
    import concourse.mybir as mybir
    import concourse.tile as tile
    import concourse.masks as masks
    from contextlib import ExitStack

    fp32 = mybir.dt.float32
    bf16 = mybir.dt.bfloat16
    Act = mybir.ActivationFunctionType
    Alu = mybir.AluOpType

    tok = b * t                    # all tokens (b-major)
    tpc = tok // NCORES            # tokens per core for proj/MLP/out
    m_qkv = 3 * HPC * D            # q, k, v local cols (384)
    kt_tiles = t // 128            # 128-token key tiles per (b,h) unit
    kt2 = kt_tiles // 2
    assert kt_tiles % 2 == 0
    qt_chunk = min(512, t)
    qt_chunks = t // qt_chunk
    n_chunk = min(512, tok)
    n_chunks = tok // n_chunk
    ck = C // 128                  # C chunks (8)
    mh_tiles = HID // 128          # hidden chunks (32)
    hg = 8                         # hidden chunks per weight-stream group
    scale = float(D) ** -0.5

    nc = bacc.Bacc("TRN2", target_bir_lowering=False, debug=False,
                   num_devices=NCORES)

    # ---- DRAM I/O ----
    xT_d = nc.dram_tensor("xT", [C, tok], bf16, kind="ExternalInput")
    wqkvT_d = nc.dram_tensor("wqkvT", [C, m_qkv], bf16, kind="ExternalInput")
    bqkv_d = nc.dram_tensor("bqkv2d", [128, 3], fp32, kind="ExternalInput")
    psw_d = nc.dram_tensor("pswd", [128, 128], bf16, kind="ExternalInput")
    cos_d = nc.dram_tensor("cosd", [128, tok], bf16, kind="ExternalInput")
    sin_d = nc.dram_tensor("sind", [128, tok], bf16, kind="ExternalInput")
    wprojT_d = nc.dram_tensor("wprojT", [C, C], bf16, kind="ExternalInput")
    bproj_d = nc.dram_tensor("bproj2d", [128, ck], fp32, kind="ExternalInput")
    w1T_d = nc.dram_tensor("w1T", [C, HID], bf16, kind="ExternalInput")
    w2T_d = nc.dram_tensor("w2T", [C, HID], bf16, kind="ExternalInput")
    w3T_d = nc.dram_tensor("w3T", [HID, C], bf16, kind="ExternalInput")
    b1_d = nc.dram_tensor("b1_2d", [128, mh_tiles], fp32, kind="ExternalInput")
    b2_d = nc.dram_tensor("b2_2d", [128, mh_tiles], fp32, kind="ExternalInput")
    b3_d = nc.dram_tensor("b3_2d", [128, ck], fp32, kind="ExternalInput")
    y_d = nc.dram_tensor("y_loc", [C, tpc], fp32, kind="ExternalOutput")

    with tile.TileContext(nc) as tc:
        es = ExitStack()
        # ---- constants / biases (live whole kernel) ----
        consts = es.enter_context(tc.tile_pool(name="consts", bufs=1))
        ident = consts.tile([128, 128], bf16, name="ident")
        masks.make_identity(nc, ident[:])
        # broadcast stationary: row 0 ones, rows 1-63 zero -> MM replicates
        # the rhs row-0 reciprocal across 65 output partitions in-mode.
        onepad = consts.tile([64, 65], bf16, name="onepad")
        nc.vector.memset(onepad[:], 0.0)
        nc.vector.memset(onepad[0:1, :], 1.0)
        psw_sb = consts.tile([128, 128], bf16, name="psw_sb")
        nc.sync.dma_start(out=psw_sb[:], in_=psw_d[:, :])
        bqkv_sb = consts.tile([128, 3], fp32, name="bqkv_sb")
        nc.sync.dma_start(out=bqkv_sb[:], in_=bqkv_d[:, :])
        bproj_sb = consts.tile([128, ck], fp32, name="bproj_sb")
        nc.sync.dma_start(out=bproj_sb[:], in_=bproj_d[:, :])
        b1_sb = consts.tile([128, mh_tiles], fp32, name="b1_sb")
        nc.sync.dma_start(out=b1_sb[:], in_=b1_d[:, :])
        b2_sb = consts.tile([128, mh_tiles], fp32, name="b2_sb")
        nc.sync.dma_start(out=b2_sb[:], in_=b2_d[:, :])
        b3_sb = consts.tile([128, ck], fp32, name="b3_sb")
        nc.sync.dma_start(out=b3_sb[:], in_=b3_d[:, :])

        # ---- attention-lifetime tensors ----
        attn_pool = es.enter_context(tc.tile_pool(name="attn", bufs=1))
        qr = attn_pool.tile([128, tok], bf16, name="qr")
        kr = attn_pool.tile([128, tok], bf16, name="kr")
        # each 65-col block: [ones col | 64 v dims] so the softmax
        # denominator lands on pso ROW 0 (lane 0 = same lane the bcast
        # matmul rhs needs -> no cross-partition DMA in the tail)
        vaug_cols = 65 * kt_tiles * b * HPC
        v_aug = attn_pool.tile([128, vaug_cols], bf16, name="v_aug")
        nc.vector.memset(v_aug[:], 1.0)
        outT_h0 = attn_pool.tile([65, tok], bf16, name="outT_h0")
        outT_h1 = attn_pool.tile([65, tok], bf16, name="outT_h1")

        # ---- A2A bounce buffers (one pair per local head) ----
        dram = es.enter_context(tc.tile_pool(name="dramp", bufs=1,
                                             space="DRAM"))
        a2a_in = []
        a2a_out = []
        for hh in range(HPC):
            ai = dram.tile([NCORES * 64, tpc], bf16, name=f"a2a_in{hh}")
            ao = dram.tile([NCORES * 64, tpc], bf16, name=f"a2a_out{hh}")
            a2a_in.append(ai)
            a2a_out.append(ao)

        # ================= Phase A: QKV GEMM + RoPE + v transpose ======
        ph_a = ExitStack()
        xt_pool = ph_a.enter_context(tc.tile_pool(name="xt", bufs=1))
        wq_pool = ph_a.enter_context(tc.tile_pool(name="wq", bufs=1))
        qkv_sb_pool = ph_a.enter_context(tc.tile_pool(name="qkvsb", bufs=1))
        rope_tab = ph_a.enter_context(tc.tile_pool(name="ropetab", bufs=1))
        ps_qkv = ph_a.enter_context(
            tc.tile_pool(name="ps_qkv", bufs=3, space="PSUM"))
        ps_sw = ph_a.enter_context(
            tc.tile_pool(name="ps_sw", bufs=2, space="PSUM"))
        ps_tr = ph_a.enter_context(
            tc.tile_pool(name="ps_tr", bufs=2, space="PSUM"))

        wq_sb = []
        for kc in range(ck):
            wq_kc = wq_pool.tile([128, m_qkv], bf16, name=f"wqkv{kc}")
            nc.sync.dma_start(out=wq_kc[:],
                              in_=wqkvT_d[128 * kc:128 * kc + 128, :])
            wq_sb.append(wq_kc)
        # xT DMA'd in n-chunk column slices (n-outer) so the first QKV
        # matmul only waits for 1MB, not the full 8MB.  The RoPE tables
        # are queued behind the first two token chunks.
        xt_sb = []
        for kc in range(ck):
            xt_kc = xt_pool.tile([128, tok], bf16, name=f"xt{kc}")
            xt_sb.append(xt_kc)
        cos_sb = rope_tab.tile([128, tok], bf16, name="cos_sb")
        sin_sb = rope_tab.tile([128, tok], bf16, name="sin_sb")
        for n in range(n_chunks):
            c0 = n * n_chunk
            for kc in range(ck):
                nc.sync.dma_start(
                    out=xt_sb[kc][:, c0:c0 + n_chunk],
                    in_=xT_d[128 * kc:128 * kc + 128, c0:c0 + n_chunk])
            if n == min(1, n_chunks - 1):
                nc.sync.dma_start(out=cos_sb[:], in_=cos_d[:, :])
                nc.sync.dma_start(out=sin_sb[:], in_=sin_d[:, :])

        q_bf = qkv_sb_pool.tile([128, tok], bf16, name="q_bf")
        k_bf = qkv_sb_pool.tile([128, tok], bf16, name="k_bf")
        v_bf = qkv_sb_pool.tile([128, tok], bf16, name="v_bf")
        qtb = qkv_sb_pool.tile([128, tok], bf16, name="qtb")
        ktb = qkv_sb_pool.tile([128, tok], bf16, name="ktb")
        ta_scr = qkv_sb_pool.tile([128, tok], bf16, name="ta_scr")

        dest = [q_bf, k_bf, v_bf]
        for n in range(n_chunks):
            c0 = n * n_chunk
            cs = slice(c0, c0 + n_chunk)
            for mi in range(3):
                ps = ps_qkv.tile([128, n_chunk], fp32, name=f"psqkv{n}_{mi}",
                                 tag="psqkv")
                for kc in range(ck):
                    nc.tensor.matmul(
                        ps[:], wq_sb[kc][:, 128 * mi:128 * mi + 128],
                        xt_sb[kc][:, cs],
                        start=(kc == 0), stop=(kc == ck - 1))
                # bias add + cast to bf16 on DVE (PSUM source)
                nc.vector.tensor_scalar(
                    dest[mi][:, cs], ps[:],
                    bqkv_sb[:, mi:mi + 1], None, Alu.add)
            # full RoPE per chunk (vector work rides under the QKV matmuls):
            # dst = src*cos + swap(src)*sin, swap via permutation matmul
            for src, tb, dst in ((q_bf, qtb, qr), (k_bf, ktb, kr)):
                psx = ps_sw.tile([128, n_chunk], fp32, name=f"psw{n}",
                                 tag="psw")
                nc.tensor.matmul(psx[:], psw_sb[:], src[:, cs],
                                 start=True, stop=True)
                nc.vector.tensor_mul(tb[:, cs], psx[:], sin_sb[:, cs])
                nc.vector.tensor_mul(ta_scr[:, cs], src[:, cs],
                                     cos_sb[:, cs])
                nc.vector.tensor_add(dst[:, cs], ta_scr[:, cs], tb[:, cs])

        # v_aug[(h,bi)] blocks: [128 ktok, 64 d] + ones col (65 stride)
        # unit order is h-major so head 0 finishes first for the early A2A.
        for h in range(HPC):
            for bi in range(b):
                u = h * b + bi
                base = u * 65 * kt_tiles
                for kt in range(kt_tiles):
                    pst = ps_tr.tile([128, 64], bf16, name=f"pst{u}_{kt}",
                                     tag="pst")
                    nc.tensor.transpose(
                        pst[:],
                        v_bf[64 * h:64 * h + 64,
                             bi * t + 128 * kt:bi * t + 128 * kt + 128],
                        ident[64 * h:64 * h + 64, 64 * h:64 * h + 64])
                    nc.vector.tensor_copy(
                        v_aug[:, base + 65 * kt + 1:base + 65 * kt + 65],
                        pst[:])

        ph_a.close()

        # ---- pools opened between phases: prefetched weights + aT ----
        es2 = ExitStack()
        wp_pool = es2.enter_context(tc.tile_pool(name="wpp", bufs=1))
        of_pool = es2.enter_context(tc.tile_pool(name="ofp", bufs=1))
        mlp_pool = es2.enter_context(tc.tile_pool(name="mlp", bufs=1))
        w1g_pool = es2.enter_context(tc.tile_pool(name="w1g", bufs=2))
        w2g_pool = es2.enter_context(tc.tile_pool(name="w2g", bufs=2))
        aT = mlp_pool.tile([128, ck * tpc], bf16, name="aT")

        wp_sb = []
        for kc in range(ck):
            wp_kc = wp_pool.tile([128, C], bf16, name=f"wp{kc}")
            nc.sync.dma_start(out=wp_kc[:],
                              in_=wprojT_d[128 * kc:128 * kc + 128, :])
            wp_sb.append(wp_kc)

        def load_mlp_group(g, defer=None):
            w1g = []
            w2g = []
            for kc in range(ck):
                w1k = w1g_pool.tile([128, hg * 128], bf16,
                                    name=f"w1g{g}_{kc}", tag=f"w1g{kc}")
                w1g.append(w1k)
                w2k = w2g_pool.tile([128, hg * 128], bf16,
                                    name=f"w2g{g}_{kc}", tag=f"w2g{kc}")
                w2g.append(w2k)

                def dma(w1k=w1k, w2k=w2k, g=g, kc=kc):
                    nc.sync.dma_start(
                        out=w1k[:],
                        in_=w1T_d[128 * kc:128 * kc + 128,
                                  hg * 128 * g:hg * 128 * (g + 1)])
                    nc.sync.dma_start(
                        out=w2k[:],
                        in_=w2T_d[128 * kc:128 * kc + 128,
                                  hg * 128 * g:hg * 128 * (g + 1)])

                if defer is None:
                    dma()
                else:
                    defer.append(dma)
            return w1g, w2g

        # both buffered groups prefetch mid-attention (emitted after the
        # first collective fires, so the h=0 slots' small per-slot DMAs
        # never ring-block behind 8MB of weights)
        mlp_groups = {}
        pending_dma = []

        # ================= Phase C: attention (pipelined) =================
        ph_c = ExitStack()
        ps_s = ph_c.enter_context(tc.tile_pool(name="ps_s", bufs=2,
                                               space="PSUM"))
        ps_o = ph_c.enter_context(tc.tile_pool(name="ps_o", bufs=2,
                                               space="PSUM"))
        exp_pool = ph_c.enter_context(tc.tile_pool(name="expp", bufs=11))
        sm_pool = ph_c.enter_context(tc.tile_pool(name="smp", bufs=2))
        ib_pool = ph_c.enter_context(tc.tile_pool(name="ibp", bufs=2))
        ss_pool = ph_c.enter_context(tc.tile_pool(name="ssp", bufs=2))

        # slots: h-major so h=0 completes first
        slots = [(h, bi, qc) for h in range(HPC) for bi in range(b)
                 for qc in range(qt_chunks)]
        nslots = len(slots)
        h0_last = nslots // 2 - 1    # index of last h=0 slot
        state = {}  # si -> (exp chunks, psoA, psoB)

        def a2a_emit(hh):
            nc.gpsimd.collective_compute(
                "AllToAll", Alu.bypass,
                replica_groups=[list(range(NCORES))],
                ins=[a2a_in[hh][:]], outs=[a2a_out[hh][:]])

        def of_load(hh):
            # kept out of the attention window: these DMAs wait on the
            # collective and would head-of-line-block the sync DMA queue
            for kc in range(ck // 2):
                of_kc = of_pool.tile([128, tpc], bf16, name=f"of{hh}_{kc}")
                nc.sync.dma_start(
                    out=of_kc[:],
                    in_=a2a_out[hh][128 * kc:128 * kc + 128, :])
                of_sb.append(of_kc)

        of_sb = []

        def sc_step(si, kp):
            h, bi, qc = slots[si]
            q0 = bi * t + qc * qt_chunk
            krows = kr[64 * h:64 * h + 64, :]
            qrows = qr[64 * h:64 * h + 64, :]
            ps = ps_s.tile([128, 2 * qt_chunk], fp32,
                           name=f"pss{si}_{kp}", tag="pss")
            for j in (0, 1):
                kt = 2 * kp + j
                nc.tensor.matmul(
                    ps[:, j * qt_chunk:(j + 1) * qt_chunk],
                    krows[:, bi * t + 128 * kt:bi * t + 128 * kt + 128],
                    qrows[:, q0:q0 + qt_chunk],
                    start=True, stop=True)
            ex = exp_pool.tile([128, 2 * qt_chunk], bf16,
                               name=f"ex{si}_{kp}", tag="ex")
            nc.scalar.activation(ex[:], ps[:], Act.Exp, scale=scale)
            state[si][0].append(ex)

        def av_step(si, kp):
            h, bi, qc = slots[si]
            u = h * b + bi
            vbase = u * 65 * kt_tiles
            if kp == 0:
                psoA = ps_o.tile([65, qt_chunk], fp32, name=f"psoA{si}",
                                 tag="psoA")
                psoB = ps_o.tile([65, qt_chunk], fp32, name=f"psoB{si}",
                                 tag="psoB")
                state[si] = (state[si][0], psoA, psoB)
            _, psoA, psoB = state[si]
            ex = state[si][0][kp]
            for j in (0, 1):
                kt = 2 * kp + j
                vcols = v_aug[:, vbase + 65 * kt:vbase + 65 * kt + 65]
                exj = ex[:, j * qt_chunk:(j + 1) * qt_chunk]
                nc.tensor.matmul(psoA[:], vcols[0:64, :], exj[0:64, :],
                                 start=(kt == 0), stop=(kt == kt_tiles - 1))
                nc.tensor.matmul(psoB[:], vcols[64:128, :], exj[64:128, :],
                                 start=(kt == 0), stop=(kt == kt_tiles - 1))

        tail_state = {}

        def tail_a(si):
            # vector chain: sum the two pso halves, reciprocal of the
            # denominator row, stage the bf16 reciprocal at partition 0
            _, psoA, psoB = state[si]
            sB = ss_pool.tile([65, qt_chunk], fp32, name=f"sB{si}", tag="sB")
            nc.vector.tensor_copy(sB[:], psoB[:])
            ssum = ss_pool.tile([65, qt_chunk], fp32, name=f"ss{si}",
                                tag="ss")
            nc.vector.tensor_add(ssum[:], psoA[:], sB[:])
            inv = sm_pool.tile([65, qt_chunk], fp32, name=f"inv{si}",
                               tag="inv")
            nc.vector.reciprocal(inv[0:1, :], ssum[0:1, :])
            invb = ib_pool.tile([64, qt_chunk], bf16, name=f"ib{si}",
                                tag="ib")
            nc.vector.memset(invb[:], 0.0)
            # denominator already on lane 0: cast-copy, no DMA needed
            nc.vector.tensor_copy(invb[0:1, :], inv[0:1, :])
            tail_state[si] = (ssum, invb)
            state.pop(si)

        def tail_b(si):
            # emitted one slot later so the broadcast matmul's vector-chain
            # dependencies are long since ready (no tensor-queue stall)
            h, bi, qc = slots[si]
            ssum, invb = tail_state.pop(si)
            psb = ps_o.tile([65, qt_chunk], fp32, name=f"psb{si}", tag="psoB")
            nc.tensor.matmul(psb[:], onepad[:], invb[:], start=True,
                             stop=True)
            out_h = outT_h0 if h == 0 else outT_h1
            q0 = bi * t + qc * qt_chunk
            # row 0 computes denom*recip (=1, unused); rows 1-64 are the
            # output. Full-range op keeps the DVE start partition at 0.
            nc.vector.tensor_mul(out_h[:, q0:q0 + qt_chunk],
                                 ssum[:, :], psb[:, :])
            # this slot's columns cover whole A2A token-blocks: ship now
            assert qt_chunk % tpc == 0
            for j in range(q0 // tpc, (q0 + qt_chunk) // tpc):
                nc.sync.dma_start(out=a2a_in[h][64 * j:64 * j + 64, :],
                                  in_=out_h[1:65, tpc * j:tpc * (j + 1)])

        def open_slot(si):
            state[si] = ([], None, None)

        # software pipeline: scores of slot si interleave with attn@v of
        # si-1; the normalize matmul of si-2 rides along a further slot late
        open_slot(0)
        for kp in range(kt2):
            sc_step(0, kp)
        for si in range(1, nslots):
            open_slot(si)
            for kp in range(kt2):
                sc_step(si, kp)
                av_step(si - 1, kp)
            tail_a(si - 1)
            if si >= 2:
                tail_b(si - 2)
                if si - 2 == h0_last:
                    a2a_emit(0)
                    mlp_groups[0] = load_mlp_group(0, defer=pending_dma)
                    mlp_groups[1] = load_mlp_group(1, defer=pending_dma)
                else:
                    # dribble the deferred weight prefetch between slots so
                    # the per-slot DMAs never ring-block behind it
                    for _ in range(3):
                        if pending_dma:
                            pending_dma.pop(0)()
        for kp in range(kt2):
            av_step(nslots - 1, kp)
        tail_a(nslots - 1)
        tail_b(nslots - 2)
        tail_b(nslots - 1)
        a2a_emit(1)
        while pending_dma:
            pending_dma.pop(0)()
        of_load(0)
        of_load(1)

        ph_c.close()

        # ================= Phase E: out proj (token-sharded) ==========
        ph_e = ExitStack()
        ps_a = ph_e.enter_context(tc.tile_pool(name="ps_a", bufs=1,
                                               space="PSUM"))
        ps_am = []
        for m in range(ck):
            psa = ps_a.tile([128, tpc], fp32, name=f"psa{m}", tag=f"psa{m}")
            ps_am.append(psa)
        for kc in range(ck):
            for m in range(ck):
                nc.tensor.matmul(ps_am[m][:],
                                 wp_sb[kc][:, 128 * m:128 * m + 128],
                                 of_sb[kc][:],
                                 start=(kc == 0), stop=(kc == ck - 1))
        for m in range(ck):
            nc.scalar.activation(aT[:, tpc * m:tpc * (m + 1)], ps_am[m][:],
                                 Act.Identity, bias=bproj_sb[:, m:m + 1])
        ph_e.close()

        # ================= Phase F: SwiGLU MLP =================
        ph_f = ExitStack()
        ps_1 = ph_f.enter_context(tc.tile_pool(name="ps_1", bufs=2,
                                               space="PSUM"))
        ps_2 = ph_f.enter_context(tc.tile_pool(name="ps_2", bufs=2,
                                               space="PSUM"))
        s_pool = ph_f.enter_context(tc.tile_pool(name="sp", bufs=2))
        hT = mlp_pool.tile([128, mh_tiles * tpc], bf16, name="hT")

        ngroups = mh_tiles // hg
        for g in range(ngroups):
            w1g, w2g = mlp_groups.pop(g)
            if g + 2 < ngroups:
                mlp_groups[g + 2] = load_mlp_group(g + 2)
            for ml in range(hg):
                mh = hg * g + ml
                ps1 = ps_1.tile([128, tpc], fp32, name=f"ps1_{mh}", tag="ps1")
                ps2 = ps_2.tile([128, tpc], fp32, name=f"ps2_{mh}", tag="ps2")
                for kc in range(ck):
                    nc.tensor.matmul(ps1[:],
                                     w1g[kc][:, 128 * ml:128 * ml + 128],
                                     aT[:, tpc * kc:tpc * (kc + 1)],
                                     start=(kc == 0), stop=(kc == ck - 1))
                for kc in range(ck):
                    nc.tensor.matmul(ps2[:],
                                     w2g[kc][:, 128 * ml:128 * ml + 128],
                                     aT[:, tpc * kc:tpc * (kc + 1)],
                                     start=(kc == 0), stop=(kc == ck - 1))
                g_sb = s_pool.tile([128, tpc], fp32, name=f"g{mh}", tag="g")
                nc.scalar.activation(g_sb[:], ps1[:], Act.Sigmoid,
                                     bias=b1_sb[:, mh:mh + 1])
                s_sb = s_pool.tile([128, tpc], fp32, name=f"s{mh}", tag="s")
                nc.vector.scalar_tensor_tensor(
                    s_sb[:], ps1[:], b1_sb[:, mh:mh + 1], g_sb[:],
                    op0=Alu.add, op1=Alu.mult)
                nc.vector.scalar_tensor_tensor(
                    hT[:, tpc * mh:tpc * (mh + 1)], ps2[:],
                    b2_sb[:, mh:mh + 1], s_sb[:],
                    op0=Alu.add, op1=Alu.mult)
        ph_f.close()

        # ---- w3 ----
        ph_g = ExitStack()
        w3_pool = ph_g.enter_context(tc.tile_pool(name="w3p", bufs=3))
        ps_3 = ph_g.enter_context(tc.tile_pool(name="ps_3", bufs=1,
                                               space="PSUM"))
        out_pool = ph_g.enter_context(tc.tile_pool(name="outp", bufs=2))
        ps_3m = []
        for m in range(ck):
            ps3 = ps_3.tile([128, tpc], fp32, name=f"ps3_{m}", tag=f"ps3{m}")
            ps_3m.append(ps3)
        for kh in range(mh_tiles):
            w3k = w3_pool.tile([128, C], bf16, name=f"w3k{kh}", tag="w3k")
            nc.sync.dma_start(out=w3k[:],
                              in_=w3T_d[128 * kh:128 * kh + 128, :])
            for m in range(ck):
                nc.tensor.matmul(ps_3m[m][:],
                                 w3k[:, 128 * m:128 * m + 128],
                                 hT[:, tpc * kh:tpc * (kh + 1)],
                                 start=(kh == 0), stop=(kh == mh_tiles - 1))
        for m in range(ck):
            yo = out_pool.tile([128, tpc], fp32, name=f"yo{m}", tag="yo")
            nc.scalar.activation(yo[:], ps_3m[m][:], Act.Identity,
                                 bias=b3_sb[:, m:m + 1])
            nc.sync.dma_start(out=y_d[128 * m:128 * m + 128, :], in_=yo[:])
        ph_g.close()
        es2.close()
        es.close()

    nc.compile()
    return nc


@functools.lru_cache(maxsize=2)
def _get_program(b, t):
    return _build_program(b, t)


def _prep_inputs(x, w_qkv, b_qkv, w_proj, b_proj, w1, b1, w2, b2, w3, b3,
                 cos, sin, b, t):
    """Build per-core in_maps (host-side sharding / transposes / casts)."""
    bf = ml_dtypes.bfloat16
    tok = b * t
    ck = C // 128
    mh_tiles = HID // 128

    xT = np.ascontiguousarray(x.reshape(tok, C).T).astype(bf)
    # RoPE tables tiled to [128, tok]: rows = 4x the 32 freq rows,
    # cols = b-major tokens.
    cosd = np.tile(cos.T, (4, b)).astype(bf)
    sind = np.tile(np.concatenate([-sin.T, sin.T], axis=0), (2, b)).astype(bf)
    # half-swap permutation matrix: out row m <- in row swap(m)
    # (swap first/last 32 within each head's 64-dim block)
    psw = np.zeros((128, 128), dtype=np.float32)
    for m in range(128):
        j = m + 32 if (m % 64) < 32 else m - 32
        psw[j, m] = 1.0
    pswd = psw.astype(bf)
    # proj weight rows reordered: even global heads first, then odd
    # (matches the two per-head A2A result layouts)
    wpT = np.ascontiguousarray(w_proj.T)         # [H*D, C]
    head_order = list(range(0, H, 2)) + list(range(1, H, 2))
    wprojT = np.concatenate(
        [wpT[h * D:(h + 1) * D, :] for h in head_order], axis=0).astype(bf)
    bproj2d = np.ascontiguousarray(b_proj.reshape(ck, 128).T).astype(np.float32)
    w1T = np.ascontiguousarray(w1.T).astype(bf)
    w2T = np.ascontiguousarray(w2.T).astype(bf)
    w3T = np.ascontiguousarray(w3.T).astype(bf)
    b1_2d = np.ascontiguousarray(b1.reshape(mh_tiles, 128).T).astype(np.float32)
    b2_2d = np.ascontiguousarray(b2.reshape(mh_tiles, 128).T).astype(np.float32)
    b3_2d = np.ascontiguousarray(b3.reshape(ck, 128).T).astype(np.float32)

    # even/odd RoPE permutation within each head's 64 dims
    perm = np.concatenate([np.arange(0, D, 2), np.arange(1, D, 2)])

    in_maps = []
    for c in range(NCORES):
        rows = []
        brows = []
        secperm = [(0, perm), (1, perm), (2, np.arange(D))]
        for sec, p in secperm:        # q, k, v
            for hh in range(HPC):
                h = HPC * c + hh
                idx = sec * H * D + h * D + p
                rows.append(w_qkv[idx, :])
                brows.append(b_qkv[idx])
        wql = np.concatenate(rows, axis=0)           # [384, C]
        bql = np.concatenate(brows, axis=0)          # [384]
        wqkvT = np.ascontiguousarray(wql.T).astype(bf)
        bqkv2d = np.ascontiguousarray(bql.reshape(3, 128).T).astype(np.float32)
        in_maps.append({
            "xT": xT, "wqkvT": wqkvT, "bqkv2d": bqkv2d, "pswd": pswd,
            "cosd": cosd, "sind": sind,
            "wprojT": wprojT, "bproj2d": bproj2d,
            "w1T": w1T, "w2T": w2T, "w3T": w3T,
            "b1_2d": b1_2d, "b2_2d": b2_2d, "b3_2d": b3_2d,
        })
    return in_maps


def kernel(x, w_qkv, b_qkv, w_proj, b_proj, w1, b1, w2, b2, w3, b3, cos, sin,
           _trace=False):
    from concourse import bass_utils

    b, t, c = x.shape
    assert (b, t, c) == (B, T, C)
    args = [np.asarray(a, dtype=np.float32) for a in
            (x, w_qkv, b_qkv, w_proj, b_proj, w1, b1, w2, b2, w3, b3,
             cos, sin)]
    nc = _get_program(b, t)
    in_maps = _prep_inputs(*args, b, t)
    res = bass_utils.run_bass_kernel_spmd(
        nc, in_maps, core_ids=list(range(NCORES)), trace=_trace)
    tpc = (b * t) // NCORES
    y = np.empty((b * t, c), dtype=np.float32)
    for i in range(NCORES):
        y[tpc * i:tpc * (i + 1), :] = res.results[i]["y_loc"].T
    out = y.reshape(b, t, c)
    if _trace:
        return out, res
    return out


# revision 48
# speedup vs baseline: 1.0621x; 1.0044x over previous
"""Trainium2 Bass kernel for a dense transformer block (RoPE attention + SwiGLU).

Sharding (8 NeuronCores, Megatron-style):
  - QKV + attention: tensor-parallel over heads (2 heads/core, both batches).
  - Two AllToAlls (one per local head) reshard attention output from
    head-sharded to token-sharded; the first is issued halfway through
    attention so it overlaps with the second head's compute.
  - proj + SwiGLU MLP: token-sharded (512 tokens/core), fully local.
Host pre-transposes x and all weights so every matmul contracts over the
partition axis. RoPE's half-swap is done on device with a permutation
matmul (avoids duplicating q/k columns in the QKV GEMM). The whole
attention phase runs in (64,128) PE-tiling mode (scores contract over
d=64; attn@v splits its 128-token contraction across the two row tiles),
so there are no PE mode-switch drains inside the phase.
Softmax: exp on ScalarE in FD=1024 chunks (the phase pacer); the
denominator reciprocal runs on DVE off the critical path, and is
broadcast across partitions with a ones-row matmul whose rhs row is
staged at partition 0 by a small DMA. The normalize matmul is emitted
one slot late so the in-order tensor queue never waits on the vector
chain. All matmuls run in bf16 with fp32 PSUM accumulation.
"""

import functools
import numpy as np
import ml_dtypes

B, T, C, H, D = 2, 2048, 1024, 16, 64
HID = 4 * C
NCORES = 8
HPC = H // NCORES          # heads per core


def _build_program(b, t):
    import concourse.bacc as bacc
    import concourse.mybir as mybir
    import concourse.tile as tile
    import concourse.masks as masks
    from contextlib import ExitStack

    fp32 = mybir.dt.float32
    bf16 = mybir.dt.bfloat16
    Act = mybir.ActivationFunctionType
    Alu = mybir.AluOpType

    tok = b * t                    # all tokens (b-major)
    tpc = tok // NCORES            # tokens per core for proj/MLP/out
    m_qkv = 3 * HPC * D            # q, k, v local cols (384)
    kt_tiles = t // 128            # 128-token key tiles per (b,h) unit
    kt2 = kt_tiles // 2
    assert kt_tiles % 2 == 0
    qt_chunk = min(512, t)
    qt_chunks = t // qt_chunk
    n_chunk = min(512, tok)
    n_chunks = tok // n_chunk
    ck = C // 128                  # C chunks (8)
    mh_tiles = HID // 128          # hidden chunks (32)
    hg = 8                         # hidden chunks per weight-stream group
    scale = float(D) ** -0.5

    nc = bacc.Bacc("TRN2", target_bir_lowering=False, debug=False,
                   num_devices=NCORES)

    # ---- DRAM I/O ----
    xT_d = nc.dram_tensor("xT", [C, tok], bf16, kind="ExternalInput")
    wqkvT_d = nc.dram_tensor("wqkvT", [C, m_qkv], bf16, kind="ExternalInput")
    bqkv_d = nc.dram_tensor("bqkv2d", [128, 3], fp32, kind="ExternalInput")
    psw_d = nc.dram_tensor("pswd", [128, 128], bf16, kind="ExternalInput")
    cos_d = nc.dram_tensor("cosd", [128, tok], bf16, kind="ExternalInput")
    sin_d = nc.dram_tensor("sind", [128, tok], bf16, kind="ExternalInput")
    wprojT_d = nc.dram_tensor("wprojT", [C, C], bf16, kind="ExternalInput")
    bproj_d = nc.dram_tensor("bproj2d", [128, ck], fp32, kind="ExternalInput")
    w1T_d = nc.dram_tensor("w1T", [C, HID], bf16, kind="ExternalInput")
    w2T_d = nc.dram_tensor("w2T", [C, HID], bf16, kind="ExternalInput")
    w3T_d = nc.dram_tensor("w3T", [HID, C], bf16, kind="ExternalInput")
    b1_d = nc.dram_tensor("b1_2d", [128, mh_tiles], fp32, kind="ExternalInput")
    b2_d = nc.dram_tensor("b2_2d", [128, mh_tiles], fp32, kind="ExternalInput")
    b3_d = nc.dram_tensor("b3_2d", [128, ck], fp32, kind="ExternalInput")
    y_d = nc.dram_tensor("y_loc", [C, tpc], fp32, kind="ExternalOutput")

    with tile.TileContext(nc) as tc:
        es = ExitStack()
        # ---- constants / biases (live whole kernel) ----
        consts = es.enter_context(tc.tile_pool(name="consts", bufs=1))
        ident = consts.tile([128, 128], bf16, name="ident")
        masks.make_identity(nc, ident[:])
        # broadcast stationary: row 0 ones, rows 1-63 zero -> MM replicates
        # the rhs row-0 reciprocal across 65 output partitions in-mode.
        onepad = consts.tile([64, 65], bf16, name="onepad")
        nc.vector.memset(onepad[:], 0.0)
        nc.vector.memset(onepad[0:1, :], 1.0)
        psw_sb = consts.tile([128, 128], bf16, name="psw_sb")
        nc.sync.dma_start(out=psw_sb[:], in_=psw_d[:, :])
        bqkv_sb = consts.tile([128, 3], fp32, name="bqkv_sb")
        nc.sync.dma_start(out=bqkv_sb[:], in_=bqkv_d[:, :])
        bproj_sb = consts.tile([128, ck], fp32, name="bproj_sb")
        nc.sync.dma_start(out=bproj_sb[:], in_=bproj_d[:, :])
        b1_sb = consts.tile([128, mh_tiles], fp32, name="b1_sb")
        nc.sync.dma_start(out=b1_sb[:], in_=b1_d[:, :])
        b2_sb = consts.tile([128, mh_tiles], fp32, name="b2_sb")
        nc.sync.dma_start(out=b2_sb[:], in_=b2_d[:, :])
        b3_sb = consts.tile([128, ck], fp32, name="b3_sb")
        nc.sync.dma_start(out=b3_sb[:], in_=b3_d[:, :])

        # ---- attention-lifetime tensors ----
        attn_pool = es.enter_context(tc.tile_pool(name="attn", bufs=1))
        qr = attn_pool.tile([128, tok], bf16, name="qr")
        kr = attn_pool.tile([128, tok], bf16, name="kr")
        # each 65-col block: [ones col | 64 v dims] so the softmax
        # denominator lands on pso ROW 0 (lane 0 = same lane the bcast
        # matmul rhs needs -> no cross-partition DMA in the tail)
        vaug_cols = 65 * kt_tiles * b * HPC
        v_aug = attn_pool.tile([128, vaug_cols], bf16, name="v_aug")
        nc.vector.memset(v_aug[:], 1.0)
        outT_h0 = attn_pool.tile([65, tok], bf16, name="outT_h0")
        outT_h1 = attn_pool.tile([65, tok], bf16, name="outT_h1")

        # ---- A2A bounce buffers (one pair per local head) ----
        dram = es.enter_context(tc.tile_pool(name="dramp", bufs=1,
                                             space="DRAM"))
        a2a_in = []
        a2a_out = []
        for hh in range(HPC):
            ai = dram.tile([NCORES * 64, tpc], bf16, name=f"a2a_in{hh}")
            ao = dram.tile([NCORES * 64, tpc], bf16, name=f"a2a_out{hh}")
            a2a_in.append(ai)
            a2a_out.append(ao)

        # ================= Phase A: QKV GEMM + RoPE + v transpose ======
        ph_a = ExitStack()
        xt_pool = ph_a.enter_context(tc.tile_pool(name="xt", bufs=1))
        wq_pool = ph_a.enter_context(tc.tile_pool(name="wq", bufs=1))
        qkv_sb_pool = ph_a.enter_context(tc.tile_pool(name="qkvsb", bufs=1))
        rope_tab = ph_a.enter_context(tc.tile_pool(name="ropetab", bufs=1))
        ps_qkv = ph_a.enter_context(
            tc.tile_pool(name="ps_qkv", bufs=3, space="PSUM"))
        ps_sw = ph_a.enter_context(
            tc.tile_pool(name="ps_sw", bufs=2, space="PSUM"))
        ps_tr = ph_a.enter_context(
            tc.tile_pool(name="ps_tr", bufs=2, space="PSUM"))

        wq_sb = []
        for kc in range(ck):
            wq_kc = wq_pool.tile([128, m_qkv], bf16, name=f"wqkv{kc}")
            nc.sync.dma_start(out=wq_kc[:],
                              in_=wqkvT_d[128 * kc:128 * kc + 128, :])
            wq_sb.append(wq_kc)
        # xT DMA'd in n-chunk column slices (n-outer) so the first QKV
        # matmul only waits for 1MB, not the full 8MB.  The RoPE tables
        # are queued behind the first two token chunks.
        xt_sb = []
        for kc in range(ck):
            xt_kc = xt_pool.tile([128, tok], bf16, name=f"xt{kc}")
            xt_sb.append(xt_kc)
        cos_sb = rope_tab.tile([128, tok], bf16, name="cos_sb")
        sin_sb = rope_tab.tile([128, tok], bf16, name="sin_sb")
        for n in range(n_chunks):
            c0 = n * n_chunk
            # first chunks ride the Activation HWDGE queue in parallel with
            # the sync-queue weight loads to shorten the startup fill
            eng = nc.scalar if n < 2 else nc.sync
            for kc in range(ck):
                eng.dma_start(
                    out=xt_sb[kc][:, c0:c0 + n_chunk],
                    in_=xT_d[128 * kc:128 * kc + 128, c0:c0 + n_chunk])
            if n == min(1, n_chunks - 1):
                nc.sync.dma_start(out=cos_sb[:], in_=cos_d[:, :])
                nc.sync.dma_start(out=sin_sb[:], in_=sin_d[:, :])

        q_bf = qkv_sb_pool.tile([128, tok], bf16, name="q_bf")
        k_bf = qkv_sb_pool.tile([128, tok], bf16, name="k_bf")
        v_bf = qkv_sb_pool.tile([128, tok], bf16, name="v_bf")
        qtb = qkv_sb_pool.tile([128, tok], bf16, name="qtb")
        ktb = qkv_sb_pool.tile([128, tok], bf16, name="ktb")
        ta_scr = qkv_sb_pool.tile([128, tok], bf16, name="ta_scr")

        dest = [q_bf, k_bf, v_bf]
        for n in range(n_chunks):
            c0 = n * n_chunk
            cs = slice(c0, c0 + n_chunk)
            for mi in range(3):
                ps = ps_qkv.tile([128, n_chunk], fp32, name=f"psqkv{n}_{mi}",
                                 tag="psqkv")
                for kc in range(ck):
                    nc.tensor.matmul(
                        ps[:], wq_sb[kc][:, 128 * mi:128 * mi + 128],
                        xt_sb[kc][:, cs],
                        start=(kc == 0), stop=(kc == ck - 1))
                # bias add + cast to bf16 on DVE (PSUM source)
                nc.vector.tensor_scalar(
                    dest[mi][:, cs], ps[:],
                    bqkv_sb[:, mi:mi + 1], None, Alu.add)
            # full RoPE per chunk (vector work rides under the QKV matmuls):
            # dst = src*cos + swap(src)*sin, swap via permutation matmul
            for src, tb, dst in ((q_bf, qtb, qr), (k_bf, ktb, kr)):
                psx = ps_sw.tile([128, n_chunk], fp32, name=f"psw{n}",
                                 tag="psw")
                nc.tensor.matmul(psx[:], psw_sb[:], src[:, cs],
                                 start=True, stop=True)
                nc.vector.tensor_mul(tb[:, cs], psx[:], sin_sb[:, cs])
                nc.vector.tensor_mul(ta_scr[:, cs], src[:, cs],
                                     cos_sb[:, cs])
                nc.vector.tensor_add(dst[:, cs], ta_scr[:, cs], tb[:, cs])

        # v_aug[(h,bi)] blocks: [128 ktok, 64 d] + ones col (65 stride)
        # unit order is h-major so head 0 finishes first for the early A2A.
        for h in range(HPC):
            for bi in range(b):
                u = h * b + bi
                base = u * 65 * kt_tiles
                for kt in range(kt_tiles):
                    pst = ps_tr.tile([128, 64], bf16, name=f"pst{u}_{kt}",
                                     tag="pst")
                    nc.tensor.transpose(
                        pst[:],
                        v_bf[64 * h:64 * h + 64,
                             bi * t + 128 * kt:bi * t + 128 * kt + 128],
                        ident[64 * h:64 * h + 64, 64 * h:64 * h + 64])
                    nc.vector.tensor_copy(
                        v_aug[:, base + 65 * kt + 1:base + 65 * kt + 65],
                        pst[:])

        ph_a.close()

        # ---- pools opened between phases: prefetched weights + aT ----
        es2 = ExitStack()
        wp_pool = es2.enter_context(tc.tile_pool(name="wpp", bufs=1))
        of_pool = es2.enter_context(tc.tile_pool(name="ofp", bufs=1))
        mlp_pool = es2.enter_context(tc.tile_pool(name="mlp", bufs=1))
        w1g_pool = es2.enter_context(tc.tile_pool(name="w1g", bufs=2))
        w2g_pool = es2.enter_context(tc.tile_pool(name="w2g", bufs=2))
        aT = mlp_pool.tile([128, ck * tpc], bf16, name="aT")

        wp_sb = []
        for kc in range(ck):
            wp_kc = wp_pool.tile([128, C], bf16, name=f"wp{kc}")
            nc.sync.dma_start(out=wp_kc[:],
                              in_=wprojT_d[128 * kc:128 * kc + 128, :])
            wp_sb.append(wp_kc)

        def load_mlp_group(g, defer=None):
            w1g = []
            w2g = []
            for kc in range(ck):
                w1k = w1g_pool.tile([128, hg * 128], bf16,
                                    name=f"w1g{g}_{kc}", tag=f"w1g{kc}")
                w1g.append(w1k)
                w2k = w2g_pool.tile([128, hg * 128], bf16,
                                    name=f"w2g{g}_{kc}", tag=f"w2g{kc}")
                w2g.append(w2k)

                def dma(w1k=w1k, w2k=w2k, g=g, kc=kc):
                    nc.sync.dma_start(
                        out=w1k[:],
                        in_=w1T_d[128 * kc:128 * kc + 128,
                                  hg * 128 * g:hg * 128 * (g + 1)])
                    nc.sync.dma_start(
                        out=w2k[:],
                        in_=w2T_d[128 * kc:128 * kc + 128,
                                  hg * 128 * g:hg * 128 * (g + 1)])

                if defer is None:
                    dma()
                else:
                    defer.append(dma)
            return w1g, w2g

        # both buffered groups prefetch mid-attention (emitted after the
        # first collective fires, so the h=0 slots' small per-slot DMAs
        # never ring-block behind 8MB of weights)
        mlp_groups = {}
        pending_dma = []

        # ================= Phase C: attention (pipelined) =================
        ph_c = ExitStack()
        ps_s = ph_c.enter_context(tc.tile_pool(name="ps_s", bufs=2,
                                               space="PSUM"))
        ps_o = ph_c.enter_context(tc.tile_pool(name="ps_o", bufs=2,
                                               space="PSUM"))
        exp_pool = ph_c.enter_context(tc.tile_pool(name="expp", bufs=11))
        sm_pool = ph_c.enter_context(tc.tile_pool(name="smp", bufs=2))
        ib_pool = ph_c.enter_context(tc.tile_pool(name="ibp", bufs=2))
        ss_pool = ph_c.enter_context(tc.tile_pool(name="ssp", bufs=2))

        # slots: h-major so h=0 completes first
        slots = [(h, bi, qc) for h in range(HPC) for bi in range(b)
                 for qc in range(qt_chunks)]
        nslots = len(slots)
        h0_last = nslots // 2 - 1    # index of last h=0 slot
        state = {}  # si -> (exp chunks, psoA, psoB)

        def a2a_emit(hh):
            nc.gpsimd.collective_compute(
                "AllToAll", Alu.bypass,
                replica_groups=[list(range(NCORES))],
                ins=[a2a_in[hh][:]], outs=[a2a_out[hh][:]])

        def of_load(hh):
            # kept out of the attention window: these DMAs wait on the
            # collective and would head-of-line-block the sync DMA queue
            for kc in range(ck // 2):
                of_kc = of_pool.tile([128, tpc], bf16, name=f"of{hh}_{kc}")
                nc.sync.dma_start(
                    out=of_kc[:],
                    in_=a2a_out[hh][128 * kc:128 * kc + 128, :])
                of_sb.append(of_kc)

        of_sb = []

        def sc_step(si, kp):
            h, bi, qc = slots[si]
            q0 = bi * t + qc * qt_chunk
            krows = kr[64 * h:64 * h + 64, :]
            qrows = qr[64 * h:64 * h + 64, :]
            ps = ps_s.tile([128, 2 * qt_chunk], fp32,
                           name=f"pss{si}_{kp}", tag="pss")
            for j in (0, 1):
                kt = 2 * kp + j
                nc.tensor.matmul(
                    ps[:, j * qt_chunk:(j + 1) * qt_chunk],
                    krows[:, bi * t + 128 * kt:bi * t + 128 * kt + 128],
                    qrows[:, q0:q0 + qt_chunk],
                    start=True, stop=True)
            ex = exp_pool.tile([128, 2 * qt_chunk], bf16,
                               name=f"ex{si}_{kp}", tag="ex")
            nc.scalar.activation(ex[:], ps[:], Act.Exp, scale=scale)
            state[si][0].append(ex)

        def av_step(si, kp):
            h, bi, qc = slots[si]
            u = h * b + bi
            vbase = u * 65 * kt_tiles
            if kp == 0:
                psoA = ps_o.tile([65, qt_chunk], fp32, name=f"psoA{si}",
                                 tag="psoA")
                psoB = ps_o.tile([65, qt_chunk], fp32, name=f"psoB{si}",
                                 tag="psoB")
                state[si] = (state[si][0], psoA, psoB)
            _, psoA, psoB = state[si]
            ex = state[si][0][kp]
            for j in (0, 1):
                kt = 2 * kp + j
                vcols = v_aug[:, vbase + 65 * kt:vbase + 65 * kt + 65]
                exj = ex[:, j * qt_chunk:(j + 1) * qt_chunk]
                nc.tensor.matmul(psoA[:], vcols[0:64, :], exj[0:64, :],
                                 start=(kt == 0), stop=(kt == kt_tiles - 1))
                nc.tensor.matmul(psoB[:], vcols[64:128, :], exj[64:128, :],
                                 start=(kt == 0), stop=(kt == kt_tiles - 1))

        tail_state = {}

        def tail_a(si):
            # vector chain: sum the two pso halves, reciprocal of the
            # denominator row, stage the bf16 reciprocal at partition 0
            _, psoA, psoB = state[si]
            sB = ss_pool.tile([65, qt_chunk], fp32, name=f"sB{si}", tag="sB")
            nc.vector.tensor_copy(sB[:], psoB[:])
            ssum = ss_pool.tile([65, qt_chunk], fp32, name=f"ss{si}",
                                tag="ss")
            nc.vector.tensor_add(ssum[:], psoA[:], sB[:])
            inv = sm_pool.tile([65, qt_chunk], fp32, name=f"inv{si}",
                               tag="inv")
            nc.vector.reciprocal(inv[0:1, :], ssum[0:1, :])
            invb = ib_pool.tile([64, qt_chunk], bf16, name=f"ib{si}",
                                tag="ib")
            nc.vector.memset(invb[:], 0.0)
            # denominator already on lane 0: cast-copy, no DMA needed
            nc.vector.tensor_copy(invb[0:1, :], inv[0:1, :])
            tail_state[si] = (ssum, invb)
            state.pop(si)

        def tail_b(si):
            # emitted one slot later so the broadcast matmul's vector-chain
            # dependencies are long since ready (no tensor-queue stall)
            h, bi, qc = slots[si]
            ssum, invb = tail_state.pop(si)
            psb = ps_o.tile([65, qt_chunk], fp32, name=f"psb{si}", tag="psoB")
            nc.tensor.matmul(psb[:], onepad[:], invb[:], start=True,
                             stop=True)
            out_h = outT_h0 if h == 0 else outT_h1
            q0 = bi * t + qc * qt_chunk
            # row 0 computes denom*recip (=1, unused); rows 1-64 are the
            # output. Full-range op keeps the DVE start partition at 0.
            nc.vector.tensor_mul(out_h[:, q0:q0 + qt_chunk],
                                 ssum[:, :], psb[:, :])
            # this slot's columns cover whole A2A token-blocks: ship now
            assert qt_chunk % tpc == 0
            for j in range(q0 // tpc, (q0 + qt_chunk) // tpc):
                nc.sync.dma_start(out=a2a_in[h][64 * j:64 * j + 64, :],
                                  in_=out_h[1:65, tpc * j:tpc * (j + 1)])

        def open_slot(si):
            state[si] = ([], None, None)

        # software pipeline: scores of slot si interleave with attn@v of
        # si-1; the normalize matmul of si-2 rides along a further slot late
        open_slot(0)
        for kp in range(kt2):
            sc_step(0, kp)
        for si in range(1, nslots):
            open_slot(si)
            for kp in range(kt2):
                sc_step(si, kp)
                av_step(si - 1, kp)
            tail_a(si - 1)
            if si >= 2:
                tail_b(si - 2)
                if si - 2 == h0_last:
                    a2a_emit(0)
                    mlp_groups[0] = load_mlp_group(0, defer=pending_dma)
                    mlp_groups[1] = load_mlp_group(1, defer=pending_dma)
                else:
                    # dribble the deferred weight prefetch between slots so
                    # the per-slot DMAs never ring-block behind it
                    for _ in range(3):
                        if pending_dma:
                            pending_dma.pop(0)()
        for kp in range(kt2):
            av_step(nslots - 1, kp)
        tail_a(nslots - 1)
        tail_b(nslots - 2)
        tail_b(nslots - 1)
        a2a_emit(1)
        while pending_dma:
            pending_dma.pop(0)()
        of_load(0)
        of_load(1)

        ph_c.close()

        # ================= Phase E: out proj (token-sharded) ==========
        ph_e = ExitStack()
        ps_a = ph_e.enter_context(tc.tile_pool(name="ps_a", bufs=1,
                                               space="PSUM"))
        # w3 streams with 3-deep lookahead, first loads issued here so
        # they arrive long before phase G needs them
        w3_pool = es2.enter_context(tc.tile_pool(name="w3p", bufs=3))
        w3_tiles = {}

        def load_w3(kh):
            w3k = w3_pool.tile([128, C], bf16, name=f"w3k{kh}", tag="w3k")
            nc.sync.dma_start(out=w3k[:],
                              in_=w3T_d[128 * kh:128 * kh + 128, :])
            w3_tiles[kh] = w3k

        for kh in range(3):
            load_w3(kh)
        ps_am = []
        for m in range(ck):
            psa = ps_a.tile([128, tpc], fp32, name=f"psa{m}", tag=f"psa{m}")
            ps_am.append(psa)
        for kc in range(ck):
            for m in range(ck):
                nc.tensor.matmul(ps_am[m][:],
                                 wp_sb[kc][:, 128 * m:128 * m + 128],
                                 of_sb[kc][:],
                                 start=(kc == 0), stop=(kc == ck - 1))
        for m in range(ck):
            nc.scalar.activation(aT[:, tpc * m:tpc * (m + 1)], ps_am[m][:],
                                 Act.Identity, bias=bproj_sb[:, m:m + 1])
        ph_e.close()

        # ================= Phase F: SwiGLU MLP =================
        ph_f = ExitStack()
        ps_1 = ph_f.enter_context(tc.tile_pool(name="ps_1", bufs=2,
                                               space="PSUM"))
        ps_2 = ph_f.enter_context(tc.tile_pool(name="ps_2", bufs=2,
                                               space="PSUM"))
        s_pool = ph_f.enter_context(tc.tile_pool(name="sp", bufs=2))
        hT = mlp_pool.tile([128, mh_tiles * tpc], bf16, name="hT")

        ngroups = mh_tiles // hg
        for g in range(ngroups):
            w1g, w2g = mlp_groups.pop(g)
            if g + 2 < ngroups:
                mlp_groups[g + 2] = load_mlp_group(g + 2)
            for ml in range(hg):
                mh = hg * g + ml
                ps1 = ps_1.tile([128, tpc], fp32, name=f"ps1_{mh}", tag="ps1")
                ps2 = ps_2.tile([128, tpc], fp32, name=f"ps2_{mh}", tag="ps2")
                for kc in range(ck):
                    nc.tensor.matmul(ps1[:],
                                     w1g[kc][:, 128 * ml:128 * ml + 128],
                                     aT[:, tpc * kc:tpc * (kc + 1)],
                                     start=(kc == 0), stop=(kc == ck - 1))
                for kc in range(ck):
                    nc.tensor.matmul(ps2[:],
                                     w2g[kc][:, 128 * ml:128 * ml + 128],
                                     aT[:, tpc * kc:tpc * (kc + 1)],
                                     start=(kc == 0), stop=(kc == ck - 1))
                g_sb = s_pool.tile([128, tpc], fp32, name=f"g{mh}", tag="g")
                nc.scalar.activation(g_sb[:], ps1[:], Act.Sigmoid,
                                     bias=b1_sb[:, mh:mh + 1])
                s_sb = s_pool.tile([128, tpc], fp32, name=f"s{mh}", tag="s")
                nc.vector.scalar_tensor_tensor(
                    s_sb[:], ps1[:], b1_sb[:, mh:mh + 1], g_sb[:],
                    op0=Alu.add, op1=Alu.mult)
                nc.vector.scalar_tensor_tensor(
                    hT[:, tpc * mh:tpc * (mh + 1)], ps2[:],
                    b2_sb[:, mh:mh + 1], s_sb[:],
                    op0=Alu.add, op1=Alu.mult)
        ph_f.close()

        # ---- w3 ----
        ph_g = ExitStack()
        ps_3 = ph_g.enter_context(tc.tile_pool(name="ps_3", bufs=1,
                                               space="PSUM"))
        out_pool = ph_g.enter_context(tc.tile_pool(name="outp", bufs=2))
        ps_3m = []
        for m in range(ck):
            ps3 = ps_3.tile([128, tpc], fp32, name=f"ps3_{m}", tag=f"ps3{m}")
            ps_3m.append(ps3)
        for kh in range(mh_tiles):
            w3k = w3_tiles.pop(kh)
            if kh + 3 < mh_tiles:
                load_w3(kh + 3)
            for m in range(ck):
                nc.tensor.matmul(ps_3m[m][:],
                                 w3k[:, 128 * m:128 * m + 128],
                                 hT[:, tpc * kh:tpc * (kh + 1)],
                                 start=(kh == 0), stop=(kh == mh_tiles - 1))
        for m in range(ck):
            yo = out_pool.tile([128, tpc], fp32, name=f"yo{m}", tag="yo")
            nc.scalar.activation(yo[:], ps_3m[m][:], Act.Identity,
                                 bias=b3_sb[:, m:m + 1])
            nc.sync.dma_start(out=y_d[128 * m:128 * m + 128, :], in_=yo[:])
        ph_g.close()
        es2.close()
        es.close()

    nc.compile()
    return nc


@functools.lru_cache(maxsize=2)
def _get_program(b, t):
    return _build_program(b, t)


def _prep_inputs(x, w_qkv, b_qkv, w_proj, b_proj, w1, b1, w2, b2, w3, b3,
                 cos, sin, b, t):
    """Build per-core in_maps (host-side sharding / transposes / casts)."""
    bf = ml_dtypes.bfloat16
    tok = b * t
    ck = C // 128
    mh_tiles = HID // 128

    xT = np.ascontiguousarray(x.reshape(tok, C).T).astype(bf)
    # RoPE tables tiled to [128, tok]: rows = 4x the 32 freq rows,
    # cols = b-major tokens.
    cosd = np.tile(cos.T, (4, b)).astype(bf)
    sind = np.tile(np.concatenate([-sin.T, sin.T], axis=0), (2, b)).astype(bf)
    # half-swap permutation matrix: out row m <- in row swap(m)
    # (swap first/last 32 within each head's 64-dim block)
    psw = np.zeros((128, 128), dtype=np.float32)
    for m in range(128):
        j = m + 32 if (m % 64) < 32 else m - 32
        psw[j, m] = 1.0
    pswd = psw.astype(bf)
    # proj weight rows reordered: even global heads first, then odd
    # (matches the two per-head A2A result layouts)
    wpT = np.ascontiguousarray(w_proj.T)         # [H*D, C]
    head_order = list(range(0, H, 2)) + list(range(1, H, 2))
    wprojT = np.concatenate(
        [wpT[h * D:(h + 1) * D, :] for h in head_order], axis=0).astype(bf)
    bproj2d = np.ascontiguousarray(b_proj.reshape(ck, 128).T).astype(np.float32)
    w1T = np.ascontiguousarray(w1.T).astype(bf)
    w2T = np.ascontiguousarray(w2.T).astype(bf)
    w3T = np.ascontiguousarray(w3.T).astype(bf)
    b1_2d = np.ascontiguousarray(b1.reshape(mh_tiles, 128).T).astype(np.float32)
    b2_2d = np.ascontiguousarray(b2.reshape(mh_tiles, 128).T).astype(np.float32)
    b3_2d = np.ascontiguousarray(b3.reshape(ck, 128).T).astype(np.float32)

    # even/odd RoPE permutation within each head's 64 dims
    perm = np.concatenate([np.arange(0, D, 2), np.arange(1, D, 2)])

    in_maps = []
    for c in range(NCORES):
        rows = []
        brows = []
        secperm = [(0, perm), (1, perm), (2, np.arange(D))]
        for sec, p in secperm:        # q, k, v
            for hh in range(HPC):
                h = HPC * c + hh
                idx = sec * H * D + h * D + p
                rows.append(w_qkv[idx, :])
                brows.append(b_qkv[idx])
        wql = np.concatenate(rows, axis=0)           # [384, C]
        bql = np.concatenate(brows, axis=0)          # [384]
        wqkvT = np.ascontiguousarray(wql.T).astype(bf)
        bqkv2d = np.ascontiguousarray(bql.reshape(3, 128).T).astype(np.float32)
        in_maps.append({
            "xT": xT, "wqkvT": wqkvT, "bqkv2d": bqkv2d, "pswd": pswd,
            "cosd": cosd, "sind": sind,
            "wprojT": wprojT, "bproj2d": bproj2d,
            "w1T": w1T, "w2T": w2T, "w3T": w3T,
            "b1_2d": b1_2d, "b2_2d": b2_2d, "b3_2d": b3_2d,
        })
    return in_maps


def kernel(x, w_qkv, b_qkv, w_proj, b_proj, w1, b1, w2, b2, w3, b3, cos, sin,
           _trace=False):
    from concourse import bass_utils

    b, t, c = x.shape
    assert (b, t, c) == (B, T, C)
    args = [np.asarray(a, dtype=np.float32) for a in
            (x, w_qkv, b_qkv, w_proj, b_proj, w1, b1, w2, b2, w3, b3,
             cos, sin)]
    nc = _get_program(b, t)
    in_maps = _prep_inputs(*args, b, t)
    res = bass_utils.run_bass_kernel_spmd(
        nc, in_maps, core_ids=list(range(NCORES)), trace=_trace)
    tpc = (b * t) // NCORES
    y = np.empty((b * t, c), dtype=np.float32)
    for i in range(NCORES):
        y[tpc * i:tpc * (i + 1), :] = res.results[i]["y_loc"].T
    out = y.reshape(b, t, c)
    if _trace:
        return out, res
    return out
